# revision 1
# baseline (speedup 1.0000x reference)
"""Trainium2 Bass kernel for nn_BitSpikeMambaModel (embed -> bitlinear x2 -> LN -> bitlinear head).

Self-contained: hardcodes shapes from the problem spec.
Sharding: pure data-parallel over the 4096 tokens (512 tokens per core, 8 cores).
Per core:
  - embedding gather via transpose-mode dma_gather from fp16 hi/lo copies of emb
    (exact fp32 = hi + lo to ~2^-21), producing xT [128, D/128, T] (d on partitions)
  - BitNet ternary quantization on device: q = Sign(w) * (|w| > 0.5*scale),
    which is exactly clip(round_half_even(w/scale), -1, 1) for these ranges.
    scale = mean|w|: w0/w1 reduced locally; head scale reduced over a per-core
    vocab slice + AllReduce across the 8 cores.
  - all matmuls in fp16 (ternary weights exact in fp16), activations optionally
    split hi+lo fp16 for near-fp32 accuracy, accumulation in fp32 PSUM.
  - LayerNorm stats via ones-matmul on the tensor engine, Newton-refined rsqrt.
  - head streamed from DRAM in 256-column groups: DMA fp32 -> quantize -> matmul.
Output per core: [V, T] (vocab-major); host reassembles/transposes to [2, S, V].
"""

import math
import numpy as np

import concourse.bass as bass
import concourse.bacc as bacc
import concourse.mybir as mybir
import concourse.tile as tile
from concourse.bass_utils import run_bass_kernel_spmd

F32 = mybir.dt.float32
F16 = mybir.dt.float16
F32R = mybir.dt.float32r
I16 = mybir.dt.int16
AF = mybir.ActivationFunctionType
OP = mybir.AluOpType
AX = mybir.AxisListType

VOCAB = 32000
DIM = 2048
BATCH = 2
SEQ = 2048
NCORES = 8
EPS = 1e-5


class Cfg:
    def __init__(self, V=VOCAB, D=DIM, T=(BATCH * SEQ) // NCORES, ncores=NCORES,
                 hi_lo=True, G=2, head_r=False):
        assert D % 128 == 0 and V % 128 == 0 and T % 128 == 0 and T <= 512
        self.V, self.D, self.T, self.ncores, self.hi_lo, self.G = V, D, T, ncores, hi_lo, G
        self.head_r = head_r  # head matmuls in float32r (trunk stays fp16 hi/lo)
        self.DT = D // 128          # d-tiles
        self.NO_TR = D // 128       # trunk output tiles
        self.NO_HD = V // 128       # head output tiles
        assert self.NO_TR % G == 0 and self.NO_HD % G == 0
        assert V % ncores == 0
        self.VS = V // ncores       # per-core vocab slice for head abs-mean


def _chunk_cols(n):
    """Largest divisor of n that is <= 256, for scale-pass streaming."""
    for c in range(min(n, 256), 0, -1):
        if n % c == 0:
            return c
    return n


def _inner_k(c):
    """Largest divisor of c that is <= 128, for the 2-level inner reduce."""
    for k in range(min(c, 128), 0, -1):
        if c % k == 0:
            return k
    return c


def build(cfg: Cfg):
    V, D, T, G, DT = cfg.V, cfg.D, cfg.T, cfg.G, cfg.DT
    nc = bacc.Bacc("TRN2", target_bir_lowering=False, debug=False,
                   num_devices=cfg.ncores)

    # ---- DRAM I/O ----
    idx_d = nc.dram_tensor("idx", [128, T // 16], I16, kind="ExternalInput")
    embh_d = nc.dram_tensor("embh", [V, D], F16, kind="ExternalInput")
    embl_d = nc.dram_tensor("embl", [V, D], F16, kind="ExternalInput")
    w0t_d = nc.dram_tensor("w0t", [D, D], F32, kind="ExternalInput")
    w1t_d = nc.dram_tensor("w1t", [D, D], F32, kind="ExternalInput")
    hwt_d = nc.dram_tensor("hwt", [D, V], F32, kind="ExternalInput")
    wsl_d = nc.dram_tensor("wsl", [D, cfg.VS], F32, kind="ExternalInput")
    b0_d = nc.dram_tensor("b0r", [128, DT], F32, kind="ExternalInput")
    b1_d = nc.dram_tensor("b1r", [128, DT], F32, kind="ExternalInput")
    gam_d = nc.dram_tensor("gamr", [128, DT], F32, kind="ExternalInput")
    bet_d = nc.dram_tensor("betr", [128, DT], F32, kind="ExternalInput")
    hb_d = nc.dram_tensor("hbr", [128, cfg.NO_HD], F32, kind="ExternalInput")
    out_d = nc.dram_tensor("out", [V, T], F32, kind="ExternalOutput")

    w0t_v = w0t_d.ap().rearrange("(dt p) o -> p dt o", p=128)
    w1t_v = w1t_d.ap().rearrange("(dt p) o -> p dt o", p=128)
    hwt_v = hwt_d.ap().rearrange("(dt p) o -> p dt o", p=128)
    wsl_v = wsl_d.ap().rearrange("(dt p) o -> p dt o", p=128)

    with tile.TileContext(nc) as tc:
        import contextlib
        with contextlib.ExitStack() as ctx:
            cst = ctx.enter_context(tc.tile_pool(name="cst", bufs=1))
            big = ctx.enter_context(tc.tile_pool(name="big", bufs=4 if cfg.hi_lo else 2))
            ybuf = ctx.enter_context(tc.tile_pool(name="ybuf", bufs=1))
            wstream = ctx.enter_context(tc.tile_pool(name="wstream", bufs=2))
            qbuf = ctx.enter_context(tc.tile_pool(name="qbuf", bufs=2))
            mbuf = ctx.enter_context(tc.tile_pool(name="mbuf", bufs=2))
            evt = ctx.enter_context(tc.tile_pool(name="evt", bufs=2))
            osb = ctx.enter_context(tc.tile_pool(name="osb", bufs=2))
            sml = ctx.enter_context(tc.tile_pool(name="sml", bufs=1))
            scl = ctx.enter_context(tc.tile_pool(name="scl", bufs=1))
            ps_mm = ctx.enter_context(tc.tile_pool(name="ps_mm", bufs=4, space="PSUM"))
            ps_st = ctx.enter_context(tc.tile_pool(name="ps_st", bufs=1, space="PSUM"))
            drp = ctx.enter_context(tc.tile_pool(name="drp", bufs=2, space="DRAM"))

            # ---- constants ----
            ones_col = cst.tile([128, 1], F32)
            nc.any.memset(ones_col[:], 1.0)
            ones_row = cst.tile([1, 128], F32)
            nc.any.memset(ones_row[:], 1.0)
            idx_sb = cst.tile([128, T // 16], I16)
            nc.sync.dma_start(idx_sb[:], idx_d.ap())
            b0s = cst.tile([128, DT], F32)
            nc.sync.dma_start(b0s[:], b0_d.ap())
            b1s = cst.tile([128, DT], F32)
            nc.sync.dma_start(b1s[:], b1_d.ap())
            gams = cst.tile([128, DT], F32)
            nc.sync.dma_start(gams[:], gam_d.ap())
            bets = cst.tile([128, DT], F32)
            nc.sync.dma_start(bets[:], bet_d.ap())
            hbs = cst.tile([128, cfg.NO_HD], F32)
            nc.sync.dma_start(hbs[:], hb_d.ap())

            # ---- abs-mean of a [D, N] DRAM view -> scalar SBUF [1,1] (sum only) ----
            def abs_sum(view, N, tagsuf):
                c = _chunk_cols(N)
                k = _inner_k(c)
                nch = N // c
                c1 = c // k
                part = scl.tile([128, DT, c1 * nch], F32, tag=f"part{tagsuf}")
                for ch in range(nch):
                    wt = wstream.tile([128, DT, 256], F32, tag="wstream")
                    nc.sync.dma_start(wt[:, :, :c], view[:, :, ch * c:(ch + 1) * c])
                    nc.vector.tensor_reduce(
                        part[:, :, ch * c1:(ch + 1) * c1],
                        wt[:, :, :c].rearrange("p dt (c1 k) -> p dt c1 k", k=k),
                        axis=AX.X, op=OP.add, apply_absolute_value=True)
                p2 = sml.tile([128, DT], F32, tag="p2")
                nc.vector.tensor_reduce(p2[:], part[:], axis=AX.X, op=OP.add)
                p3 = sml.tile([128, 1], F32, tag="p3")
                nc.vector.tensor_reduce(p3[:], p2[:], axis=AX.X, op=OP.add)
                tps = ps_st.tile([1, 1], F32, tag="pa")
                nc.tensor.matmul(tps[:], ones_col[:], p3[:], start=True, stop=True)
                tot = sml.tile([1, 1], F32, tag=f"tot{tagsuf}")
                nc.scalar.activation(tot[:], tps[:], AF.Copy)
                return tot

            # scalar [1,1] -> replicated [128,1] * mul, then s=max(s,EPS), h=0.5*s
            def finalize_scale(tot, mul, tagsuf):
                rps = ps_st.tile([128, 1], F32, tag="pa")
                nc.tensor.matmul(rps[:], ones_row[:], tot[:], start=True, stop=True)
                s = scl.tile([128, 1], F32, tag=f"s{tagsuf}")
                nc.scalar.activation(s[:], rps[:], AF.Copy, scale=mul)
                nc.vector.tensor_scalar(s[:], s[:], EPS, None, OP.max)
                h = scl.tile([128, 1], F32, tag=f"h{tagsuf}")
                nc.vector.tensor_scalar(h[:], s[:], 0.5, None, OP.mult)
                nh = scl.tile([128, 1], F32, tag=f"nh{tagsuf}")
                nc.vector.tensor_scalar(nh[:], h[:], -1.0, None, OP.mult)
                return s, h, nh

            # ---- w0 scale (critical path for L0) + gather, overlapped ----
            tot0 = abs_sum(w0t_v, D, "w0")

            # ---- embedding gather (transpose mode) ----
            xt_hi = big.tile([128, DT, T], F16, tag="big")
            nc.gpsimd.dma_gather(out_ap=xt_hi[:], in_ap=embh_d.ap(), idxs_ap=idx_sb[:],
                                 num_idxs=T, num_idxs_reg=T, elem_size=D, transpose=True)
            if cfg.hi_lo:
                xt_lo = big.tile([128, DT, T], F16, tag="big")
                nc.gpsimd.dma_gather(out_ap=xt_lo[:], in_ap=embl_d.ap(), idxs_ap=idx_sb[:],
                                     num_idxs=T, num_idxs_reg=T, elem_size=D, transpose=True)

            s0, h0, nh0 = finalize_scale(tot0, 1.0 / (D * D), "w0")

            # ---- generic streamed bitlinear: for each group of G o-tiles:
            #      DMA w fp32 -> quantize -> G x (DT matmuls + evict) ----
            def bitlinear(wview, n_otiles, h_ap, nh_ap, rhs_hi, rhs_lo, consume,
                          qdt=F16, b_on_gpsimd=False):
                for g in range(n_otiles // G):
                    wt = wstream.tile([128, DT, 256], F32, tag="wstream")
                    nc.sync.dma_start(wt[:, :, :G * 128],
                                      wview[:, :, g * G * 128:(g + 1) * G * 128])
                    wt = wt[:, :, :G * 128]
                    # ternary q = 1{w > h} - 1{w < -h}  (== clip(round_half_even(w/s)))
                    sgn = qbuf.tile([128, DT, G * 128], qdt, tag="sgn")
                    nc.vector.tensor_scalar(sgn[:], wt, h_ap[:], None, OP.is_gt)
                    msk = mbuf.tile([128, DT, G * 128], F16, tag="msk")
                    eng = nc.gpsimd if b_on_gpsimd else nc.vector
                    eng.tensor_scalar(msk[:], wt, nh_ap[:], -1.0,
                                      OP.is_lt, OP.mult)
                    nc.vector.tensor_tensor(sgn[:], sgn[:], msk[:], OP.add)  # in-place q
                    for j in range(G):
                        ot = g * G + j
                        pt = ps_mm.tile([128, T], F32, tag="ps_mm")
                        n_acc = DT * (2 if rhs_lo is not None else 1)
                        i = 0
                        for dt in range(DT):
                            lhsT = sgn[:, dt, j * 128:(j + 1) * 128]
                            nc.tensor.matmul(pt[:], lhsT, rhs_hi[:, dt, :],
                                             start=(i == 0), stop=(i == n_acc - 1))
                            i += 1
                            if rhs_lo is not None:
                                nc.tensor.matmul(pt[:], lhsT, rhs_lo[:, dt, :],
                                                 start=False, stop=(i == n_acc - 1))
                                i += 1
                        consume(ot, pt)

            # ---- layer 0 ----
            h1hi = big.tile([128, DT, T], F16, tag="big")
            h1lo = big.tile([128, DT, T], F16, tag="big", name="h1lo") if cfg.hi_lo else None

            def consume_l0(ot, pt):
                if cfg.hi_lo:
                    tmp = evt.tile([128, T], F32, tag="evt")
                    nc.scalar.activation(tmp[:], pt[:], AF.Identity,
                                         bias=b0s[:, ot:ot + 1], scale=s0[:])
                    nc.vector.tensor_copy(h1hi[:, ot, :], tmp[:])
                    nc.vector.tensor_tensor(h1lo[:, ot, :], tmp[:], h1hi[:, ot, :],
                                            OP.subtract)
                else:
                    nc.scalar.activation(h1hi[:, ot, :], pt[:], AF.Identity,
                                         bias=b0s[:, ot:ot + 1], scale=s0[:])

            bitlinear(w0t_v, cfg.NO_TR, h0, nh0, xt_hi, xt_lo if cfg.hi_lo else None,
                      consume_l0)

            # ---- w1 + head scales, emitted here so their DMA/DVE overlap L0/L1 ----
            tot1 = abs_sum(w1t_v, D, "w1")
            s1, h1, nh1 = finalize_scale(tot1, 1.0 / (D * D), "w1")
            toth = abs_sum(wsl_v, cfg.VS, "hd")
            bin_t = drp.tile([1, 1], F32)
            bout_t = drp.tile([1, 1], F32)
            nc.sync.dma_start(bin_t[:], toth[:])
            nc.gpsimd.collective_compute(
                "AllReduce", OP.add,
                replica_groups=[list(range(cfg.ncores))],
                ins=[bin_t[:].opt()], outs=[bout_t[:].opt()])
            toth_g = sml.tile([1, 1], F32, tag="tothg")
            nc.sync.dma_start(toth_g[:], bout_t[:])
            sh, hh, nhh = finalize_scale(toth_g, 1.0 / (D * V), "hd")

            # ---- layer 1 (keep full fp32 output for LN) ----
            y1 = ybuf.tile([128, DT, T], F32, tag="y1")
            ps_s = ps_st.tile([1, T], F32, tag="ps_s")
            ps_q = ps_st.tile([1, T], F32, tag="ps_q")

            def consume_l1(ot, pt):
                y1out = y1[:, ot, :].bitcast(F32R) if cfg.head_r else y1[:, ot, :]
                nc.scalar.activation(y1out, pt[:], AF.Identity,
                                     bias=b1s[:, ot:ot + 1], scale=s1[:])
                sq = evt.tile([128, T], F32, tag="evt")
                nc.vector.tensor_tensor(sq[:], y1[:, ot, :], y1[:, ot, :], OP.mult)
                nc.tensor.matmul(ps_s[:], ones_col[:], y1[:, ot, :],
                                 start=(ot == 0), stop=(ot == DT - 1))
                nc.tensor.matmul(ps_q[:], ones_col[:], sq[:],
                                 start=(ot == 0), stop=(ot == DT - 1))

            bitlinear(w1t_v, cfg.NO_TR, h1, nh1, h1hi, h1lo, consume_l1)

            # ---- layernorm ----
            mu = sml.tile([1, T], F32, tag="mu")
            nc.scalar.activation(mu[:], ps_s[:], AF.Copy, scale=1.0 / D)
            ms = sml.tile([1, T], F32, tag="ms")
            nc.scalar.activation(ms[:], ps_q[:], AF.Copy, scale=1.0 / D)
            var = sml.tile([1, T], F32, tag="var")
            nc.vector.tensor_tensor(var[:], mu[:], mu[:], OP.mult)
            nc.vector.tensor_tensor(var[:], ms[:], var[:], OP.subtract)
            eps1 = cst.tile([1, 1], F32)
            nc.any.memset(eps1[:], EPS)
            sd = sml.tile([1, T], F32, tag="sd")
            nc.scalar.activation(sd[:], var[:], AF.Sqrt, bias=eps1[:])
            r0 = sml.tile([1, T], F32, tag="r0")
            nc.vector.reciprocal(r0[:], sd[:])
            # one Newton step: r = r0 * (1.5 - 0.5 * (var+eps) * r0^2)
            ve = sml.tile([1, T], F32, tag="ms")
            nc.vector.tensor_scalar(ve[:], var[:], EPS, None, OP.add)
            r2 = sml.tile([1, T], F32, tag="sd")
            nc.vector.tensor_tensor(r2[:], r0[:], r0[:], OP.mult)
            nc.vector.tensor_tensor(r2[:], ve[:], r2[:], OP.mult)
            nc.vector.tensor_scalar(r2[:], r2[:], -0.5, 1.5, OP.mult, OP.add)
            rstd = sml.tile([1, T], F32, tag="rstd")
            nc.vector.tensor_tensor(rstd[:], r0[:], r2[:], OP.mult)
            negmur = sml.tile([1, T], F32, tag="r0")
            nc.vector.tensor_tensor(negmur[:], mu[:], rstd[:], OP.mult)
            nc.vector.tensor_scalar(negmur[:], negmur[:], -1.0, None, OP.mult)
            # broadcast to [128, T] via ones-matmul
            pa = ps_st.tile([128, T], F32, tag="pa")
            nc.tensor.matmul(pa[:], ones_row[:], rstd[:], start=True, stop=True)
            a_b = cst.tile([128, T], F32)
            nc.scalar.activation(a_b[:], pa[:], AF.Copy)
            pb = ps_st.tile([128, T], F32, tag="pa")
            nc.tensor.matmul(pb[:], ones_row[:], negmur[:], start=True, stop=True)
            b_b = cst.tile([128, T], F32)
            nc.scalar.activation(b_b[:], pb[:], AF.Copy)

            if cfg.head_r:
                # write the LN output back into y1's buffer, rounded to fp32r
                h3hi = h3lo = None
                for dt in range(DT):
                    t1 = evt.tile([128, T], F32, tag="evt")
                    nc.vector.tensor_tensor(t1[:], y1[:, dt, :], a_b[:], OP.mult)
                    nc.vector.tensor_tensor(t1[:], t1[:], b_b[:], OP.add)
                    nc.vector.tensor_scalar(t1[:], t1[:], gams[:, dt:dt + 1],
                                            bets[:, dt:dt + 1], OP.mult, OP.add)
                    nc.vector.tensor_copy(y1[:, dt, :].bitcast(F32R), t1[:])
                h3r = y1[:].bitcast(F32R)
            else:
                h3hi = big.tile([128, DT, T], F16, tag="big")
                h3lo = big.tile([128, DT, T], F16, tag="big", name="h3lo") if cfg.hi_lo else None
                for dt in range(DT):
                    t1 = evt.tile([128, T], F32, tag="evt")
                    nc.vector.tensor_tensor(t1[:], y1[:, dt, :], a_b[:], OP.mult)
                    nc.vector.tensor_tensor(t1[:], t1[:], b_b[:], OP.add)
                    nc.vector.tensor_scalar(t1[:], t1[:], gams[:, dt:dt + 1],
                                            bets[:, dt:dt + 1], OP.mult, OP.add)
                    nc.vector.tensor_copy(h3hi[:, dt, :], t1[:])
                    if cfg.hi_lo:
                        nc.vector.tensor_tensor(h3lo[:, dt, :], t1[:], h3hi[:, dt, :],
                                                OP.subtract)

            # ---- head ----
            def consume_head(ot, pt):
                o = osb.tile([128, T], F32, tag="osb")
                nc.scalar.activation(o[:], pt[:], AF.Identity,
                                     bias=hbs[:, ot:ot + 1], scale=sh[:])
                nc.sync.dma_start(out_d.ap()[ot * 128:(ot + 1) * 128, :], o[:])

            if cfg.head_r:
                bitlinear(hwt_v, cfg.NO_HD, hh, nhh, h3r, None, consume_head,
                          qdt=F32R)
            else:
                bitlinear(hwt_v, cfg.NO_HD, hh, nhh, h3hi, h3lo, consume_head)

    nc.compile()
    return nc


_BUILD_CACHE = {}


def _get_nc(cfg: Cfg):
    key = (cfg.V, cfg.D, cfg.T, cfg.ncores, cfg.hi_lo, cfg.G, cfg.head_r)
    if key not in _BUILD_CACHE:
        _BUILD_CACHE[key] = build(cfg)
    return _BUILD_CACHE[key]


def make_in_maps(cfg: Cfg, x, emb, w0, b0, w1, b1, ln_gamma, ln_beta, head_w, head_b):
    """Host-side sharding/layout prep. Returns list of per-core input dicts."""
    V, D, T = cfg.V, cfg.D, cfg.T
    emb = np.asarray(emb, np.float32)
    embh = emb.astype(np.float16)
    embl = (emb - embh.astype(np.float32)).astype(np.float16)
    w0t = np.ascontiguousarray(np.asarray(w0, np.float32).T)
    w1t = np.ascontiguousarray(np.asarray(w1, np.float32).T)
    hwt = np.ascontiguousarray(np.asarray(head_w, np.float32).T)

    def rearr(v, n):
        return np.ascontiguousarray(np.asarray(v, np.float32).reshape(n, 128).T)

    b0r = rearr(b0, D // 128)
    b1r = rearr(b1, D // 128)
    gamr = rearr(ln_gamma, D // 128)
    betr = rearr(ln_beta, D // 128)
    hbr = rearr(head_b, V // 128)

    ids = np.asarray(x).reshape(-1).astype(np.int16)
    assert ids.size == cfg.ncores * T
    in_maps = []
    for c in range(cfg.ncores):
        # indices wrapped into 16 partitions, replicated across the 8 Q7 stripes
        idx_arr = np.tile(ids[c * T:(c + 1) * T].reshape(T // 16, 16).T, (8, 1))
        wsl = np.ascontiguousarray(hwt[:, c * cfg.VS:(c + 1) * cfg.VS])
        in_maps.append(dict(
            idx=idx_arr, embh=embh, embl=embl, w0t=w0t, w1t=w1t, hwt=hwt,
            wsl=wsl, b0r=b0r, b1r=b1r, gamr=gamr, betr=betr, hbr=hbr))
    return in_maps


def _run(cfg: Cfg, inputs, trace=False):
    nc = _get_nc(cfg)
    in_maps = make_in_maps(cfg, **inputs)
    res = run_bass_kernel_spmd(nc, in_maps, core_ids=list(range(cfg.ncores)),
                               trace=trace)
    outs = [res.results[c]["out"].reshape(cfg.V, cfg.T) for c in range(cfg.ncores)]
    full = np.concatenate([o.T for o in outs], axis=0)  # [ncores*T, V]
    return full, res


def kernel(**inputs) -> np.ndarray:
    cfg = Cfg()
    full, _ = _run(cfg, inputs)
    return full.reshape(BATCH, SEQ, VOCAB)



# revision 8
# speedup vs baseline: 1.6463x; 1.6463x over previous
"""Trainium2 Bass kernel for nn_BitSpikeMambaModel (embed -> bitlinear x2 -> LN -> bitlinear head).

Self-contained: hardcodes shapes from the problem spec.

Sharding:
  - trunk (embed, L0, L1, LN): data-parallel over the 4096 tokens
    (512 tokens per core, 8 cores), weights replicated.
  - head bitlinear: tensor-parallel over the vocab dim. Each core owns a
    ~4000-row slice of head_w, computes logits for ALL 4096 tokens against
    its slice after an AllGather of the LN activations.

Per core:
  - embedding gather via transpose-mode dma_gather from an fp16 copy of emb,
    producing xT [128, D/128, T] (d on partitions).
  - BitNet ternary quantization on device, fused to 2 DVE ops producing -q:
    qneg = 1{w<-h} - 1{w>h} with h = 0.5*scale (equals -clip(
    round_half_even(w/scale), -1, 1) for these ranges); PSUM evictions use
    scale=-s to restore the sign. Weights stream as fp32 so the threshold
    compares match the fp32 reference exactly (fp16-rounded weights flip
    ~4e-5 of the decisions, alone costing 3e-2 absmax-rel - too much).
    All quant elementwise runs on DVE; the Pool engine tensor_scalar path
    measured ~64us per group (27x slower than DVE) - never use it.
  - scale = mean|w| computed distributed: each core abs-sums a shard
    (w0/w1: its 256-col shard on DVE; head: its vocab slice via Act-engine
    |w| + PE ones-matmul accumulation, software-pipelined into L0/L1
    weight streaming) and the totals are AllReduce'd (w0/w1 early, head at
    end of L1).
  - all matmuls in fp16 (ternary weights exact in fp16), fp32 PSUM accum.
  - LayerNorm stats via ones-matmuls (mean from h2, mean-square via
    AF.Square eviction of the same PSUM), Newton-refined rsqrt; LN applied
    as 2 DVE ops + gamma/beta on the Act engine, cast to fp16, AllGather'd.
  - head streamed in 128-col fp32 groups: quantize -> 8 x (16 matmuls into
    a 512-token PSUM tile) -> fp16 eviction with bias+scale -> one DMA per
    o-tile covering all 4096 tokens.
Output per core: [4096, 4096] fp16 (vocab-slice rows x all tokens,
zero-padded rows for cores owning 31 tiles); host reassembles to [2,S,V] f32.
"""

import numpy as np

import concourse.bass as bass
import concourse.bacc as bacc
import concourse.mybir as mybir
import concourse.tile as tile
from concourse.bass_utils import run_bass_kernel_spmd

F32 = mybir.dt.float32
F16 = mybir.dt.float16
I16 = mybir.dt.int16
AF = mybir.ActivationFunctionType
OP = mybir.AluOpType
AX = mybir.AxisListType

VOCAB = 32000
DIM = 2048
BATCH = 2
SEQ = 2048
NCORES = 8
EPS = 1e-5

T = (BATCH * SEQ) // NCORES       # 512 local tokens per core
TF = BATCH * SEQ                  # 4096 total tokens
DT = DIM // 128                   # 16 d-tiles
NV = VOCAB // 128                 # 250 head o-tiles total
HT = 32                           # head o-tiles per core (padded)
SL = DIM // NCORES                # 256-col shard for w0/w1 abs-sum
HC = 16                           # head abs-sum chunks (256 cols each)

# per-core head tile assignment: cores 0,1 get 32 tiles, cores 2..7 get 31
_CNT = [32, 32] + [31] * 6
_START = np.concatenate([[0], np.cumsum(_CNT)[:-1]]).tolist()
assert sum(_CNT) == NV


class Cfg:
    def __init__(self, G=2):
        self.G = G                # o-tile group size for trunk weight streaming


def build(cfg: Cfg):
    G = cfg.G
    nc = bacc.Bacc("TRN2", target_bir_lowering=False, debug=False,
                   num_devices=NCORES)

    # ---- DRAM I/O ----
    idx_d = nc.dram_tensor("idx", [128, T // 16], I16, kind="ExternalInput")
    embh_d = nc.dram_tensor("embh", [VOCAB, DIM], F16, kind="ExternalInput")
    w0t_d = nc.dram_tensor("w0t", [DIM, DIM], F32, kind="ExternalInput")
    w1t_d = nc.dram_tensor("w1t", [DIM, DIM], F32, kind="ExternalInput")
    wsl0_d = nc.dram_tensor("wsl0", [DIM, SL], F32, kind="ExternalInput")
    wsl1_d = nc.dram_tensor("wsl1", [DIM, SL], F32, kind="ExternalInput")
    wslh_d = nc.dram_tensor("wslh", [DIM, HT * 128], F32, kind="ExternalInput")
    b0_d = nc.dram_tensor("b0r", [128, DT], F32, kind="ExternalInput")
    b1_d = nc.dram_tensor("b1r", [128, DT], F32, kind="ExternalInput")
    gam_d = nc.dram_tensor("gamr", [128, DT], F32, kind="ExternalInput")
    bet_d = nc.dram_tensor("betr", [128, DT], F32, kind="ExternalInput")
    hb_d = nc.dram_tensor("hbr", [128, HT], F32, kind="ExternalInput")
    out_d = nc.dram_tensor("out", [HT * 128, TF], F16, kind="ExternalOutput")

    w0t_v = w0t_d.ap().rearrange("(dt p) o -> p dt o", p=128)
    w1t_v = w1t_d.ap().rearrange("(dt p) o -> p dt o", p=128)
    wsl0_v = wsl0_d.ap().rearrange("(dt p) o -> p dt o", p=128)
    wsl1_v = wsl1_d.ap().rearrange("(dt p) o -> p dt o", p=128)
    wslh_v = wslh_d.ap().rearrange("(dt p) o -> p dt o", p=128)
    groups = [list(range(NCORES))]

    with tile.TileContext(nc) as tc:
        import contextlib
        with contextlib.ExitStack() as ctx:
            cst = ctx.enter_context(tc.tile_pool(name="cst", bufs=1))
            sml = ctx.enter_context(tc.tile_pool(name="sml", bufs=1))
            scl = ctx.enter_context(tc.tile_pool(name="scl", bufs=1))
            evt = ctx.enter_context(tc.tile_pool(name="evt", bufs=2))
            drp = ctx.enter_context(tc.tile_pool(name="drp", bufs=1, space="DRAM"))
            ps_mm = ctx.enter_context(tc.tile_pool(name="ps_mm", bufs=4, space="PSUM"))

            # collective buffers (internal DRAM; outputs Shared for HBM-HBM path)
            ar1_in = drp.tile([1, 2], F32, tag="ar1i")
            ar1_out = drp.tile([1, 2], F32, tag="ar1o", addr_space="Shared")
            ar2_in = drp.tile([1, 1], F32, tag="ar2i")
            ar2_out = drp.tile([1, 1], F32, tag="ar2o", addr_space="Shared")
            ag_in = drp.tile([128, DT * T], F16, tag="agi")
            ag_out = drp.tile([NCORES, 128, DT * T], F16, tag="ago",
                              addr_space="Shared")

            # ---- constants ----
            ones_col = cst.tile([128, 1], F16)
            nc.any.memset(ones_col[:], 1.0)
            ones_colf = cst.tile([128, 1], F32)
            nc.any.memset(ones_colf[:], 1.0)
            ones_row = cst.tile([1, 128], F32)
            nc.any.memset(ones_row[:], 1.0)
            eps1 = cst.tile([1, 1], F32)
            nc.any.memset(eps1[:], EPS)
            idx_sb = cst.tile([128, T // 16], I16)
            nc.sync.dma_start(idx_sb[:], idx_d.ap())
            b0s = cst.tile([128, DT], F32)
            nc.sync.dma_start(b0s[:], b0_d.ap())
            b1s = cst.tile([128, DT], F32)
            nc.sync.dma_start(b1s[:], b1_d.ap())
            gams = cst.tile([128, DT], F32)
            nc.sync.dma_start(gams[:], gam_d.ap())
            bets = cst.tile([128, DT], F32)
            nc.sync.dma_start(bets[:], bet_d.ap())
            hbs = cst.tile([128, HT], F32)
            nc.sync.dma_start(hbs[:], hb_d.ap())

            with contextlib.ExitStack() as trunk_ctx:
                big = trunk_ctx.enter_context(tc.tile_pool(name="big", bufs=1))
                wstream = trunk_ctx.enter_context(tc.tile_pool(name="wstream", bufs=2))
                wsch = trunk_ctx.enter_context(tc.tile_pool(name="wsch", bufs=2))
                qbuf = trunk_ctx.enter_context(tc.tile_pool(name="qbuf", bufs=2))
                abuf = trunk_ctx.enter_context(tc.tile_pool(name="abuf", bufs=2))
                ps_st = trunk_ctx.enter_context(
                    tc.tile_pool(name="ps_st", bufs=1, space="PSUM"))

                # [128,1] partial sums -> [1,1] total via ones-matmul
                def sum_tail(p3, tagsuf):
                    tps = ps_st.tile([1, 1], F32, tag="pa")
                    nc.tensor.matmul(tps[:], ones_colf[:], p3[:], start=True,
                                     stop=True)
                    tot = sml.tile([1, 1], F32, tag=f"tot{tagsuf}")
                    nc.scalar.activation(tot[:], tps[:], AF.Copy)
                    return tot

                # [1,1] total -> sneg=-max(mean,EPS) [128,1], h=s/2, -h
                def finalize_scale(tot_sb, mul, tagsuf):
                    rps = ps_st.tile([128, 1], F32, tag="pa")
                    nc.tensor.matmul(rps[:], ones_row[:], tot_sb[:], start=True,
                                     stop=True)
                    s = scl.tile([128, 1], F32, tag=f"s{tagsuf}")
                    nc.scalar.activation(s[:], rps[:], AF.Copy, scale=mul)
                    nc.vector.tensor_scalar(s[:], s[:], EPS, None, OP.max)
                    sneg = scl.tile([128, 1], F32, tag=f"sn{tagsuf}")
                    nc.vector.tensor_scalar(sneg[:], s[:], -1.0, None, OP.mult)
                    h = scl.tile([128, 1], F32, tag=f"h{tagsuf}")
                    nc.vector.tensor_scalar(h[:], s[:], 0.5, None, OP.mult)
                    nh = scl.tile([128, 1], F32, tag=f"nh{tagsuf}")
                    nc.vector.tensor_scalar(nh[:], h[:], -1.0, None, OP.mult)
                    return sneg, h, nh

                # ---- local abs-sums for w0/w1 shards -> AllReduce #1 ----
                def shard_abs_sum(view, tagsuf):
                    wt = wsch.tile([128, DT, 256], F32, tag="wsh")
                    nc.sync.dma_start(wt[:], view[:])
                    part = sml.tile([128, DT, 2], F32, tag=f"part{tagsuf}")
                    nc.vector.tensor_reduce(
                        part[:], wt[:].rearrange("p dt (c k) -> p dt c k", k=128),
                        axis=AX.X, op=OP.add, apply_absolute_value=True)
                    p2 = sml.tile([128, DT], F32, tag=f"p2{tagsuf}")
                    nc.vector.tensor_reduce(p2[:], part[:], axis=AX.X, op=OP.add)
                    p3 = sml.tile([128, 1], F32, tag=f"p3{tagsuf}")
                    nc.vector.tensor_reduce(p3[:], p2[:], axis=AX.X, op=OP.add)
                    return sum_tail(p3, tagsuf)

                tot0 = shard_abs_sum(wsl0_v, "w0")
                tot1 = shard_abs_sum(wsl1_v, "w1")
                sums01 = sml.tile([1, 2], F32, tag="sums01")
                nc.vector.tensor_copy(sums01[:, 0:1], tot0[:])
                nc.vector.tensor_copy(sums01[:, 1:2], tot1[:])
                nc.sync.dma_start(ar1_in[:], sums01[:])
                nc.gpsimd.collective_compute(
                    "AllReduce", OP.add, replica_groups=groups,
                    ins=[ar1_in[:]], outs=[ar1_out[:]])

                # ---- embedding gather (after AR1 trigger on the Pool queue) ----
                xt = big.tile([128, DT, T], F16, tag="xt")
                nc.gpsimd.dma_gather(out_ap=xt[:], in_ap=embh_d.ap(),
                                     idxs_ap=idx_sb[:], num_idxs=T,
                                     num_idxs_reg=T, elem_size=DIM,
                                     transpose=True)

                sums01_g = sml.tile([1, 2], F32, tag="sums01g")
                nc.sync.dma_start(sums01_g[:], ar1_out[:])
                sn0, h0, nh0 = finalize_scale(sums01_g[:, 0:1], 1.0 / (DIM * DIM), "w0")
                sn1, h1, nh1 = finalize_scale(sums01_g[:, 1:2], 1.0 / (DIM * DIM), "w1")

                # head abs-sum accumulator: |w| chunks via Act engine, summed by
                # PE ones-matmuls into one PSUM bank across all 16 chunks
                hsum = ps_st.tile([1, 256], F32, tag="hsum")

                def head_scale_chunk(ch):
                    wt = wsch.tile([128, DT, 256], F32, tag="wsh")
                    nc.scalar.dma_start(wt[:], wslh_v[:, :, ch * 256:(ch + 1) * 256])
                    ab = abuf.tile([128, DT, 256], F16, tag="ab")
                    nc.scalar.activation(ab[:], wt[:], AF.Abs)
                    for dt in range(DT):
                        nc.tensor.matmul(hsum[:], ones_col[:], ab[:, dt, :],
                                         start=(ch == 0 and dt == 0),
                                         stop=(ch == HC - 1 and dt == DT - 1))

                # ---- streamed bitlinear for trunk ----
                def bitlinear(wview, h_ap, nh_ap, rhs, consume, chunk0):
                    for g in range(DT // G):
                        head_scale_chunk(chunk0 + g)
                        wt = wstream.tile([128, DT, G * 128], F32, tag="wstream")
                        nc.sync.dma_start(wt[:], wview[:, :, g * G * 128:(g + 1) * G * 128])
                        sgn = qbuf.tile([128, DT, G * 128], F16, tag="sgn")
                        nc.vector.tensor_scalar(sgn[:], wt[:], h_ap[:], None, OP.is_gt)
                        qng = qbuf.tile([128, DT, G * 128], F16, tag="qng")
                        nc.vector.scalar_tensor_tensor(qng[:], wt[:], nh_ap[:], sgn[:],
                                                       OP.is_lt, OP.subtract)
                        for j in range(G):
                            ot = g * G + j
                            pt = ps_mm.tile([128, T], F32, tag="ps_mm")
                            for dt in range(DT):
                                nc.tensor.matmul(pt[:], qng[:, dt, j * 128:(j + 1) * 128],
                                                 rhs[:, dt, :],
                                                 start=(dt == 0), stop=(dt == DT - 1))
                            consume(ot, pt)

                # ---- layer 0 ----
                h1sb = big.tile([128, DT, T], F16, tag="h1sb")

                def consume_l0(ot, pt):
                    nc.scalar.activation(h1sb[:, ot, :], pt[:], AF.Identity,
                                         bias=b0s[:, ot:ot + 1], scale=sn0[:])

                bitlinear(w0t_v, h0, nh0, xt, consume_l0, chunk0=0)

                # ---- layer 1 + LN stats ----
                h2sb = big.tile([128, DT, T], F16, tag="h2sb")
                ps_s = ps_st.tile([1, T], F32, tag="ps_s")
                ps_q = ps_st.tile([1, T], F32, tag="ps_q")

                def consume_l1(ot, pt):
                    nc.scalar.activation(h2sb[:, ot, :], pt[:], AF.Identity,
                                         bias=b1s[:, ot:ot + 1], scale=sn1[:])
                    sq = evt.tile([128, T], F16, tag="evt")
                    nc.scalar.activation(sq[:], pt[:], AF.Square,
                                         bias=b1s[:, ot:ot + 1], scale=sn1[:])
                    nc.tensor.matmul(ps_s[:], ones_col[:], h2sb[:, ot, :],
                                     start=(ot == 0), stop=(ot == DT - 1))
                    nc.tensor.matmul(ps_q[:], ones_col[:], sq[:],
                                     start=(ot == 0), stop=(ot == DT - 1))

                bitlinear(w1t_v, h1, nh1, h1sb, consume_l1, chunk0=DT // G)

                # head abs-sum tail -> AllReduce #2
                toth = sml.tile([1, 1], F32, tag="toth")
                nc.vector.tensor_reduce(toth[:], hsum[:], axis=AX.X, op=OP.add)
                nc.scalar.dma_start(ar2_in[:], toth[:])
                nc.gpsimd.collective_compute(
                    "AllReduce", OP.add, replica_groups=groups,
                    ins=[ar2_in[:]], outs=[ar2_out[:]])
                toth_g = sml.tile([1, 1], F32, tag="tothg")
                nc.sync.dma_start(toth_g[:], ar2_out[:])
                snh, hh, nhh = finalize_scale(toth_g, 1.0 / (DIM * VOCAB), "hd")

                # ---- layernorm ----
                mu = sml.tile([1, T], F32, tag="mu")
                nc.scalar.activation(mu[:], ps_s[:], AF.Copy, scale=1.0 / DIM)
                ms = sml.tile([1, T], F32, tag="ms")
                nc.scalar.activation(ms[:], ps_q[:], AF.Copy, scale=1.0 / DIM)
                var = sml.tile([1, T], F32, tag="var")
                nc.vector.tensor_tensor(var[:], mu[:], mu[:], OP.mult)
                nc.vector.tensor_tensor(var[:], ms[:], var[:], OP.subtract)
                sd = sml.tile([1, T], F32, tag="sd")
                nc.scalar.activation(sd[:], var[:], AF.Sqrt, bias=eps1[:])
                r0 = sml.tile([1, T], F32, tag="r0")
                nc.vector.reciprocal(r0[:], sd[:])
                # one Newton step: r = r0 * (1.5 - 0.5 * (var+eps) * r0^2)
                ve = sml.tile([1, T], F32, tag="ms")
                nc.vector.tensor_scalar(ve[:], var[:], EPS, None, OP.add)
                r2 = sml.tile([1, T], F32, tag="sd")
                nc.vector.tensor_tensor(r2[:], r0[:], r0[:], OP.mult)
                nc.vector.tensor_tensor(r2[:], ve[:], r2[:], OP.mult)
                nc.vector.tensor_scalar(r2[:], r2[:], -0.5, 1.5, OP.mult, OP.add)
                rstd = sml.tile([1, T], F32, tag="rstd")
                nc.vector.tensor_tensor(rstd[:], r0[:], r2[:], OP.mult)
                negmur = sml.tile([1, T], F32, tag="r0")
                nc.vector.tensor_tensor(negmur[:], mu[:], rstd[:], OP.mult)
                nc.vector.tensor_scalar(negmur[:], negmur[:], -1.0, None, OP.mult)
                # broadcast to [128, T] via ones-matmul
                pa = ps_st.tile([128, T], F32, tag="pa")
                nc.tensor.matmul(pa[:], ones_row[:], rstd[:], start=True, stop=True)
                a_b = cst.tile([128, T], F32)
                nc.scalar.activation(a_b[:], pa[:], AF.Copy)
                pb = ps_st.tile([128, T], F32, tag="pa")
                nc.tensor.matmul(pb[:], ones_row[:], negmur[:], start=True, stop=True)
                b_b = cst.tile([128, T], F32)
                nc.scalar.activation(b_b[:], pb[:], AF.Copy)

                # apply LN -> fp16 (scale/shift on DVE, gamma/beta on Act)
                agsb = big.tile([128, DT, T], F16, tag="agsb")
                for dt in range(DT):
                    t1 = evt.tile([128, T], F32, tag="evtf")
                    nc.vector.tensor_tensor(t1[:], h2sb[:, dt, :], a_b[:], OP.mult)
                    nc.vector.tensor_tensor(t1[:], t1[:], b_b[:], OP.add)
                    nc.scalar.activation(agsb[:, dt, :], t1[:], AF.Identity,
                                         bias=bets[:, dt:dt + 1],
                                         scale=gams[:, dt:dt + 1])
                nc.sync.dma_start(ag_in[:], agsb[:].rearrange("p dt t -> p (dt t)"))
                nc.gpsimd.collective_compute(
                    "AllGather", OP.bypass, replica_groups=groups,
                    ins=[ag_in[:]], outs=[ag_out[:]])

            # ---- head phase ----
            with contextlib.ExitStack() as head_ctx:
                acts_p = head_ctx.enter_context(tc.tile_pool(name="acts", bufs=1))
                hws = head_ctx.enter_context(tc.tile_pool(name="hws", bufs=2))
                hqb = head_ctx.enter_context(tc.tile_pool(name="hqb", bufs=2))
                osb = head_ctx.enter_context(tc.tile_pool(name="osb", bufs=2))

                acts = acts_p.tile([128, NCORES, DT, T], F16, tag="acts")
                nc.scalar.dma_start(
                    acts[:], ag_out[:].rearrange("r p f -> p r f"))

                for g in range(HT):
                    wt = hws.tile([128, DT, 128], F32, tag="hws")
                    nc.sync.dma_start(wt[:], wslh_v[:, :, g * 128:(g + 1) * 128])
                    sgn = hqb.tile([128, DT, 128], F16, tag="sgn")
                    nc.vector.tensor_scalar(sgn[:], wt[:], hh[:], None, OP.is_gt)
                    qng = hqb.tile([128, DT, 128], F16, tag="qng")
                    nc.vector.scalar_tensor_tensor(qng[:], wt[:], nhh[:], sgn[:],
                                                   OP.is_lt, OP.subtract)
                    o = osb.tile([128, NCORES, T], F16, tag="osb")
                    for r in range(NCORES):
                        pt = ps_mm.tile([128, T], F32, tag="ps_mm")
                        for dt in range(DT):
                            nc.tensor.matmul(pt[:], qng[:, dt, :], acts[:, r, dt, :],
                                             start=(dt == 0), stop=(dt == DT - 1))
                        nc.scalar.activation(o[:, r, :], pt[:], AF.Identity,
                                             bias=hbs[:, g:g + 1], scale=snh[:])
                    nc.sync.dma_start(
                        out_d.ap()[g * 128:(g + 1) * 128, :],
                        o[:].rearrange("p r t -> p (r t)"))

    nc.compile()
    return nc


_BUILD_CACHE = {}


def _get_nc(cfg: Cfg):
    key = (cfg.G,)
    if key not in _BUILD_CACHE:
        _BUILD_CACHE[key] = build(cfg)
    return _BUILD_CACHE[key]


def make_in_maps(cfg: Cfg, x, emb, w0, b0, w1, b1, ln_gamma, ln_beta, head_w, head_b):
    """Host-side sharding/layout prep. Returns list of per-core input dicts."""
    embh = np.asarray(emb, np.float32).astype(np.float16)
    w0t = np.ascontiguousarray(np.asarray(w0, np.float32).T)
    w1t = np.ascontiguousarray(np.asarray(w1, np.float32).T)
    hwt = np.ascontiguousarray(np.asarray(head_w, np.float32).T)  # [D, V]

    def rearr(v, n):
        return np.ascontiguousarray(np.asarray(v, np.float32).reshape(n, 128).T)

    b0r = rearr(b0, DT)
    b1r = rearr(b1, DT)
    gamr = rearr(ln_gamma, DT)
    betr = rearr(ln_beta, DT)
    hb = np.asarray(head_b, np.float32)

    ids = np.asarray(x).reshape(-1).astype(np.int16)
    assert ids.size == NCORES * T
    in_maps = []
    for c in range(NCORES):
        # indices wrapped into 16 partitions, replicated across the 8 Q7 stripes
        idx_arr = np.tile(ids[c * T:(c + 1) * T].reshape(T // 16, 16).T, (8, 1))
        lo, cnt = _START[c] * 128, _CNT[c] * 128
        wslh = np.zeros((DIM, HT * 128), np.float32)
        wslh[:, :cnt] = hwt[:, lo:lo + cnt]
        hbr = np.zeros((HT * 128,), np.float32)
        hbr[:cnt] = hb[lo:lo + cnt]
        in_maps.append(dict(
            idx=idx_arr, embh=embh, w0t=w0t, w1t=w1t,
            wsl0=np.ascontiguousarray(w0t[:, c * SL:(c + 1) * SL]),
            wsl1=np.ascontiguousarray(w1t[:, c * SL:(c + 1) * SL]),
            wslh=wslh, b0r=b0r, b1r=b1r, gamr=gamr, betr=betr,
            hbr=rearr(hbr, HT)))
    return in_maps


def _run(cfg: Cfg, inputs, trace=False):
    nc = _get_nc(cfg)
    in_maps = make_in_maps(cfg, **inputs)
    res = run_bass_kernel_spmd(nc, in_maps, core_ids=list(range(NCORES)),
                               trace=trace)
    full = np.empty((TF, VOCAB), np.float32)
    for c in range(NCORES):
        o = res.results[c]["out"].reshape(HT * 128, TF)
        lo, cnt = _START[c] * 128, _CNT[c] * 128
        full[:, lo:lo + cnt] = o[:cnt].T
    return full, res


def kernel(**inputs) -> np.ndarray:
    cfg = Cfg()
    full, _ = _run(cfg, inputs)
    return full.reshape(BATCH, SEQ, VOCAB)


# revision 10
# speedup vs baseline: 1.6820x; 1.0217x over previous
"""Trainium2 Bass kernel for nn_BitSpikeMambaModel (embed -> bitlinear x2 -> LN -> bitlinear head).

Self-contained: hardcodes shapes from the problem spec.

Sharding:
  - trunk (embed, L0, L1, LN): data-parallel over the 4096 tokens
    (512 tokens per core, 8 cores), weights replicated.
  - head bitlinear: tensor-parallel over the vocab dim. Each core owns a
    ~4000-row slice of head_w, computes logits for ALL 4096 tokens against
    its slice after an AllGather of the LN activations.

Per core:
  - embedding gather via transpose-mode dma_gather from an fp16 copy of emb,
    producing xT [128, D/128, T] (d on partitions).
  - BitNet ternary quantization on device, fused to 2 DVE ops producing -q:
    qneg = 1{w<-h} - 1{w>h} with h = 0.5*scale (equals -clip(
    round_half_even(w/scale), -1, 1) for these ranges); PSUM evictions use
    scale=-s to restore the sign. Weights stream as fp32 so the threshold
    compares match the fp32 reference exactly (fp16-rounded weights flip
    ~4e-5 of the decisions, alone costing 3e-2 absmax-rel - too much).
    All quant elementwise runs on DVE; the Pool engine tensor_scalar path
    measured ~64us per group (27x slower than DVE) - never use it.
  - scale = mean|w| computed distributed: each core abs-sums a shard
    (w0/w1: its 256-col shard on DVE; head: its vocab slice via Act-engine
    |w| + PE ones-matmul accumulation, software-pipelined into L0/L1
    weight streaming) and the totals are AllReduce'd (w0/w1 early, head at
    end of L1).
  - all matmuls in fp16 (ternary weights exact in fp16), fp32 PSUM accum.
  - LayerNorm stats via ones-matmuls (mean from h2, mean-square via
    AF.Square eviction of the same PSUM), Newton-refined rsqrt; LN applied
    as 2 DVE ops + gamma/beta on the Act engine, cast to fp16, AllGather'd.
  - head streamed in 128-col fp32 groups: quantize -> 8 x (16 matmuls into
    a 512-token PSUM tile) -> fp16 eviction with bias+scale -> one DMA per
    o-tile covering all 4096 tokens.
Output per core: [4096, 4096] fp16 (vocab-slice rows x all tokens,
zero-padded rows for cores owning 31 tiles); host reassembles to [2,S,V] f32.
"""

import numpy as np

import concourse.bass as bass
import concourse.bacc as bacc
import concourse.mybir as mybir
import concourse.tile as tile
from concourse.bass_utils import run_bass_kernel_spmd

F32 = mybir.dt.float32
F16 = mybir.dt.float16
I16 = mybir.dt.int16
AF = mybir.ActivationFunctionType
OP = mybir.AluOpType
AX = mybir.AxisListType

VOCAB = 32000
DIM = 2048
BATCH = 2
SEQ = 2048
NCORES = 8
EPS = 1e-5

T = (BATCH * SEQ) // NCORES       # 512 local tokens per core
TF = BATCH * SEQ                  # 4096 total tokens
DT = DIM // 128                   # 16 d-tiles
NV = VOCAB // 128                 # 250 head o-tiles total
HT = 32                           # head o-tiles per core (padded)
SL = DIM // NCORES                # 256-col shard for w0/w1 abs-sum
HC = 16                           # head abs-sum chunks (256 cols each)

# per-core head tile assignment: cores 0,1 get 32 tiles, cores 2..7 get 31
_CNT = [32, 32] + [31] * 6
_START = np.concatenate([[0], np.cumsum(_CNT)[:-1]]).tolist()
assert sum(_CNT) == NV


class Cfg:
    def __init__(self, G=2):
        self.G = G                # o-tile group size for trunk weight streaming


def build(cfg: Cfg):
    G = cfg.G
    nc = bacc.Bacc("TRN2", target_bir_lowering=False, debug=False,
                   num_devices=NCORES)

    # ---- DRAM I/O ----
    idx_d = nc.dram_tensor("idx", [128, T // 16], I16, kind="ExternalInput")
    embh_d = nc.dram_tensor("embh", [VOCAB, DIM], F16, kind="ExternalInput")
    w0t_d = nc.dram_tensor("w0t", [DIM, DIM], F32, kind="ExternalInput")
    w1t_d = nc.dram_tensor("w1t", [DIM, DIM], F32, kind="ExternalInput")
    wsl0_d = nc.dram_tensor("wsl0", [DIM, SL], F32, kind="ExternalInput")
    wsl1_d = nc.dram_tensor("wsl1", [DIM, SL], F32, kind="ExternalInput")
    wslh_d = nc.dram_tensor("wslh", [DIM, HT * 128], F32, kind="ExternalInput")
    b0_d = nc.dram_tensor("b0r", [128, DT], F32, kind="ExternalInput")
    b1_d = nc.dram_tensor("b1r", [128, DT], F32, kind="ExternalInput")
    gam_d = nc.dram_tensor("gamr", [128, DT], F32, kind="ExternalInput")
    bet_d = nc.dram_tensor("betr", [128, DT], F32, kind="ExternalInput")
    hb_d = nc.dram_tensor("hbr", [128, HT], F32, kind="ExternalInput")
    out_d = nc.dram_tensor("out", [HT * 128, TF], F16, kind="ExternalOutput")

    w0t_v = w0t_d.ap().rearrange("(dt p) o -> p dt o", p=128)
    w1t_v = w1t_d.ap().rearrange("(dt p) o -> p dt o", p=128)
    wsl0_v = wsl0_d.ap().rearrange("(dt p) o -> p dt o", p=128)
    wsl1_v = wsl1_d.ap().rearrange("(dt p) o -> p dt o", p=128)
    wslh_v = wslh_d.ap().rearrange("(dt p) o -> p dt o", p=128)
    groups = [list(range(NCORES))]

    with tile.TileContext(nc) as tc:
        import contextlib
        with contextlib.ExitStack() as ctx:
            cst = ctx.enter_context(tc.tile_pool(name="cst", bufs=1))
            sml = ctx.enter_context(tc.tile_pool(name="sml", bufs=1))
            scl = ctx.enter_context(tc.tile_pool(name="scl", bufs=1))
            evt = ctx.enter_context(tc.tile_pool(name="evt", bufs=2))
            drp = ctx.enter_context(tc.tile_pool(name="drp", bufs=1, space="DRAM"))
            ps_mm = ctx.enter_context(tc.tile_pool(name="ps_mm", bufs=3, space="PSUM"))

            # collective buffers (internal DRAM; outputs Shared for HBM-HBM path)
            ar1_in = drp.tile([1, 2], F32, tag="ar1i")
            ar1_out = drp.tile([1, 2], F32, tag="ar1o", addr_space="Shared")
            ar2_in = drp.tile([1, 1], F32, tag="ar2i")
            ar2_out = drp.tile([1, 1], F32, tag="ar2o", addr_space="Shared")
            QAG = 4
            DTQ = DT // QAG
            ag_in_q = [drp.tile([128, DTQ * T], F16, tag=f"agi{i}",
                                name=f"agi{i}") for i in range(QAG)]
            ag_out_q = [drp.tile([NCORES, 128, DTQ * T], F16, tag=f"ago{i}",
                                 name=f"ago{i}", addr_space="Shared")
                        for i in range(QAG)]

            # ---- constants ----
            ones_col = cst.tile([128, 1], F16)
            nc.any.memset(ones_col[:], 1.0)
            ones_colf = cst.tile([128, 1], F32)
            nc.any.memset(ones_colf[:], 1.0)
            ones_row = cst.tile([1, 128], F32)
            nc.any.memset(ones_row[:], 1.0)
            eps1 = cst.tile([1, 1], F32)
            nc.any.memset(eps1[:], EPS)
            idx_sb = cst.tile([128, T // 16], I16)
            nc.sync.dma_start(idx_sb[:], idx_d.ap())
            b0s = cst.tile([128, DT], F32)
            nc.scalar.dma_start(b0s[:], b0_d.ap())
            b1s = cst.tile([128, DT], F32)
            nc.scalar.dma_start(b1s[:], b1_d.ap())
            gams = cst.tile([128, DT], F32)
            nc.scalar.dma_start(gams[:], gam_d.ap())
            bets = cst.tile([128, DT], F32)
            nc.scalar.dma_start(bets[:], bet_d.ap())
            hbs = cst.tile([128, HT], F32)
            nc.scalar.dma_start(hbs[:], hb_d.ap())

            with contextlib.ExitStack() as trunk_ctx:
                big = trunk_ctx.enter_context(tc.tile_pool(name="big", bufs=1))
                wstream = trunk_ctx.enter_context(tc.tile_pool(name="wstream", bufs=3))
                wsch = trunk_ctx.enter_context(tc.tile_pool(name="wsch", bufs=2))
                qbuf = trunk_ctx.enter_context(tc.tile_pool(name="qbuf", bufs=2))
                abuf = trunk_ctx.enter_context(tc.tile_pool(name="abuf", bufs=2))
                ps_st = trunk_ctx.enter_context(
                    tc.tile_pool(name="ps_st", bufs=1, space="PSUM"))

                # [128,1] partial sums -> [1,1] total via ones-matmul
                def sum_tail(p3, tagsuf):
                    tps = ps_st.tile([1, 1], F32, tag="pa")
                    nc.tensor.matmul(tps[:], ones_colf[:], p3[:], start=True,
                                     stop=True)
                    tot = sml.tile([1, 1], F32, tag=f"tot{tagsuf}")
                    nc.scalar.activation(tot[:], tps[:], AF.Copy)
                    return tot

                # [1,1] total -> sneg=-max(mean,EPS) [128,1], h=s/2, -h
                def finalize_scale(tot_sb, mul, tagsuf):
                    rps = ps_st.tile([128, 1], F32, tag="pa")
                    nc.tensor.matmul(rps[:], ones_row[:], tot_sb[:], start=True,
                                     stop=True)
                    s = scl.tile([128, 1], F32, tag=f"s{tagsuf}")
                    nc.scalar.activation(s[:], rps[:], AF.Copy, scale=mul)
                    nc.vector.tensor_scalar(s[:], s[:], EPS, None, OP.max)
                    sneg = scl.tile([128, 1], F32, tag=f"sn{tagsuf}")
                    nc.vector.tensor_scalar(sneg[:], s[:], -1.0, None, OP.mult)
                    h = scl.tile([128, 1], F32, tag=f"h{tagsuf}")
                    nc.vector.tensor_scalar(h[:], s[:], 0.5, None, OP.mult)
                    nh = scl.tile([128, 1], F32, tag=f"nh{tagsuf}")
                    nc.vector.tensor_scalar(nh[:], h[:], -1.0, None, OP.mult)
                    return sneg, h, nh

                # ---- local abs-sums for w0/w1 shards -> AllReduce #1 ----
                def shard_abs_sum(view, tagsuf):
                    wt = wsch.tile([128, DT, 256], F32, tag="wsh")
                    nc.sync.dma_start(wt[:], view[:])
                    part = sml.tile([128, DT, 2], F32, tag=f"part{tagsuf}")
                    nc.vector.tensor_reduce(
                        part[:], wt[:].rearrange("p dt (c k) -> p dt c k", k=128),
                        axis=AX.X, op=OP.add, apply_absolute_value=True)
                    p2 = sml.tile([128, DT], F32, tag=f"p2{tagsuf}")
                    nc.vector.tensor_reduce(p2[:], part[:], axis=AX.X, op=OP.add)
                    p3 = sml.tile([128, 1], F32, tag=f"p3{tagsuf}")
                    nc.vector.tensor_reduce(p3[:], p2[:], axis=AX.X, op=OP.add)
                    return sum_tail(p3, tagsuf)

                tot0 = shard_abs_sum(wsl0_v, "w0")
                tot1 = shard_abs_sum(wsl1_v, "w1")
                sums01 = sml.tile([1, 2], F32, tag="sums01")
                nc.vector.tensor_copy(sums01[:, 0:1], tot0[:])
                nc.vector.tensor_copy(sums01[:, 1:2], tot1[:])
                nc.sync.dma_start(ar1_in[:], sums01[:])
                nc.gpsimd.collective_compute(
                    "AllReduce", OP.add, replica_groups=groups,
                    ins=[ar1_in[:]], outs=[ar1_out[:]])

                # ---- embedding gather (after AR1 trigger on the Pool queue) ----
                xt = big.tile([128, DT, T], F16, tag="xt")
                nc.gpsimd.dma_gather(out_ap=xt[:], in_ap=embh_d.ap(),
                                     idxs_ap=idx_sb[:], num_idxs=T,
                                     num_idxs_reg=T, elem_size=DIM,
                                     transpose=True)

                sums01_g = sml.tile([1, 2], F32, tag="sums01g")
                nc.scalar.dma_start(sums01_g[:], ar1_out[:])
                sn0, h0, nh0 = finalize_scale(sums01_g[:, 0:1], 1.0 / (DIM * DIM), "w0")
                sn1, h1, nh1 = finalize_scale(sums01_g[:, 1:2], 1.0 / (DIM * DIM), "w1")

                # head abs-sum accumulator: |w| chunks via Act engine, summed by
                # PE ones-matmuls into one PSUM bank across all 16 chunks
                hsum = ps_st.tile([1, 256], F32, tag="hsum")

                def head_scale_chunk(ch):
                    wt = wsch.tile([128, DT, 256], F32, tag="wsh")
                    nc.scalar.dma_start(wt[:], wslh_v[:, :, ch * 256:(ch + 1) * 256])
                    ab = abuf.tile([128, DT, 256], F16, tag="ab")
                    nc.scalar.activation(ab[:], wt[:], AF.Abs)
                    for dt in range(DT):
                        nc.tensor.matmul(hsum[:], ones_col[:], ab[:, dt, :],
                                         start=(ch == 0 and dt == 0),
                                         stop=(ch == HC - 1 and dt == DT - 1))

                # ---- streamed bitlinear for trunk ----
                def bitlinear(wview, h_ap, nh_ap, rhs, consume, chunk0):
                    for g in range(DT // G):
                        head_scale_chunk(chunk0 + g)
                        wt = wstream.tile([128, DT, G * 128], F32, tag="wstream")
                        nc.sync.dma_start(wt[:], wview[:, :, g * G * 128:(g + 1) * G * 128])
                        sgn = qbuf.tile([128, DT, G * 128], F16, tag="sgn")
                        nc.vector.tensor_scalar(sgn[:], wt[:], h_ap[:], None, OP.is_gt)
                        qng = qbuf.tile([128, DT, G * 128], F16, tag="qng")
                        nc.vector.scalar_tensor_tensor(qng[:], wt[:], nh_ap[:], sgn[:],
                                                       OP.is_lt, OP.subtract)
                        for j in range(G):
                            ot = g * G + j
                            pt = ps_mm.tile([128, T], F32, tag="ps_mm")
                            for dt in range(DT):
                                nc.tensor.matmul(pt[:], qng[:, dt, j * 128:(j + 1) * 128],
                                                 rhs[:, dt, :],
                                                 start=(dt == 0), stop=(dt == DT - 1))
                            consume(ot, pt)

                # ---- layer 0 ----
                h1sb = big.tile([128, DT, T], F16, tag="h1sb")

                def consume_l0(ot, pt):
                    nc.scalar.activation(h1sb[:, ot, :], pt[:], AF.Identity,
                                         bias=b0s[:, ot:ot + 1], scale=sn0[:])

                bitlinear(w0t_v, h0, nh0, xt, consume_l0, chunk0=0)

                # ---- layer 1 + LN stats ----
                h2sb = big.tile([128, DT, T], F16, tag="h2sb")
                ps_s = ps_st.tile([1, T], F32, tag="ps_s")
                ps_q = ps_st.tile([1, T], F32, tag="ps_q")

                def consume_l1(ot, pt):
                    nc.scalar.activation(h2sb[:, ot, :], pt[:], AF.Identity,
                                         bias=b1s[:, ot:ot + 1], scale=sn1[:])
                    sq = evt.tile([128, T], F16, tag="evt")
                    nc.scalar.activation(sq[:], pt[:], AF.Square,
                                         bias=b1s[:, ot:ot + 1], scale=sn1[:])
                    nc.tensor.matmul(ps_s[:], ones_col[:], h2sb[:, ot, :],
                                     start=(ot == 0), stop=(ot == DT - 1))
                    nc.tensor.matmul(ps_q[:], ones_col[:], sq[:],
                                     start=(ot == 0), stop=(ot == DT - 1))

                bitlinear(w1t_v, h1, nh1, h1sb, consume_l1, chunk0=DT // G)

                # head abs-sum tail -> AllReduce #2
                toth = sml.tile([1, 1], F32, tag="toth")
                nc.vector.tensor_reduce(toth[:], hsum[:], axis=AX.X, op=OP.add)
                nc.scalar.dma_start(ar2_in[:], toth[:])
                nc.gpsimd.collective_compute(
                    "AllReduce", OP.add, replica_groups=groups,
                    ins=[ar2_in[:]], outs=[ar2_out[:]])
                toth_g = sml.tile([1, 1], F32, tag="tothg")
                nc.scalar.dma_start(toth_g[:], ar2_out[:])
                snh, hh, nhh = finalize_scale(toth_g, 1.0 / (DIM * VOCAB), "hd")

                # ---- layernorm ----
                mu = sml.tile([1, T], F32, tag="mu")
                nc.scalar.activation(mu[:], ps_s[:], AF.Copy, scale=1.0 / DIM)
                ms = sml.tile([1, T], F32, tag="ms")
                nc.scalar.activation(ms[:], ps_q[:], AF.Copy, scale=1.0 / DIM)
                var = sml.tile([1, T], F32, tag="var")
                nc.vector.tensor_tensor(var[:], mu[:], mu[:], OP.mult)
                nc.vector.tensor_tensor(var[:], ms[:], var[:], OP.subtract)
                sd = sml.tile([1, T], F32, tag="sd")
                nc.scalar.activation(sd[:], var[:], AF.Sqrt, bias=eps1[:])
                r0 = sml.tile([1, T], F32, tag="r0")
                nc.vector.reciprocal(r0[:], sd[:])
                # one Newton step: r = r0 * (1.5 - 0.5 * (var+eps) * r0^2)
                ve = sml.tile([1, T], F32, tag="ms")
                nc.vector.tensor_scalar(ve[:], var[:], EPS, None, OP.add)
                r2 = sml.tile([1, T], F32, tag="sd")
                nc.vector.tensor_tensor(r2[:], r0[:], r0[:], OP.mult)
                nc.vector.tensor_tensor(r2[:], ve[:], r2[:], OP.mult)
                nc.vector.tensor_scalar(r2[:], r2[:], -0.5, 1.5, OP.mult, OP.add)
                rstd = sml.tile([1, T], F32, tag="rstd")
                nc.vector.tensor_tensor(rstd[:], r0[:], r2[:], OP.mult)
                negmur = sml.tile([1, T], F32, tag="r0")
                nc.vector.tensor_tensor(negmur[:], mu[:], rstd[:], OP.mult)
                nc.vector.tensor_scalar(negmur[:], negmur[:], -1.0, None, OP.mult)
                # broadcast to [128, T] via ones-matmul
                pa = ps_st.tile([128, T], F32, tag="pa")
                nc.tensor.matmul(pa[:], ones_row[:], rstd[:], start=True, stop=True)
                a_b = cst.tile([128, T], F32)
                nc.scalar.activation(a_b[:], pa[:], AF.Copy)
                pb = ps_st.tile([128, T], F32, tag="pa")
                nc.tensor.matmul(pb[:], ones_row[:], negmur[:], start=True, stop=True)
                b_b = cst.tile([128, T], F32)
                nc.scalar.activation(b_b[:], pb[:], AF.Copy)

                # apply LN -> fp16 (scale/shift on DVE, gamma/beta on Act),
                # reusing xt's buffer; AllGather pipelined in dt-quarters
                for dt in range(DT):
                    t1 = evt.tile([128, T], F32, tag="evtf")
                    nc.vector.tensor_tensor(t1[:], h2sb[:, dt, :], a_b[:], OP.mult)
                    nc.vector.tensor_tensor(t1[:], t1[:], b_b[:], OP.add)
                    nc.scalar.activation(xt[:, dt, :], t1[:], AF.Identity,
                                         bias=bets[:, dt:dt + 1],
                                         scale=gams[:, dt:dt + 1])
                    if dt % DTQ == DTQ - 1:
                        i = dt // DTQ
                        nc.sync.dma_start(
                            ag_in_q[i][:],
                            xt[:, i * DTQ:(i + 1) * DTQ, :].rearrange(
                                "p dt t -> p (dt t)"))
                        nc.gpsimd.collective_compute(
                            "AllGather", OP.bypass, replica_groups=groups,
                            ins=[ag_in_q[i][:]], outs=[ag_out_q[i][:]])

            # ---- head phase ----
            with contextlib.ExitStack() as head_ctx:
                acts_p = head_ctx.enter_context(tc.tile_pool(name="acts", bufs=1))
                hws = head_ctx.enter_context(tc.tile_pool(name="hws", bufs=2))
                hqb = head_ctx.enter_context(tc.tile_pool(name="hqb", bufs=2))
                osb = head_ctx.enter_context(tc.tile_pool(name="osb", bufs=2))
                ps_h = head_ctx.enter_context(
                    tc.tile_pool(name="ps_h", bufs=5, space="PSUM"))

                acts = acts_p.tile([128, NCORES, DT, T], F16, tag="acts")
                for i in range(QAG):
                    nc.scalar.dma_start(
                        acts[:, :, i * DTQ:(i + 1) * DTQ, :],
                        ag_out_q[i][:].rearrange("r p f -> p r f"))

                for g in range(HT):
                    wt = hws.tile([128, DT, 128], F32, tag="hws")
                    nc.sync.dma_start(wt[:], wslh_v[:, :, g * 128:(g + 1) * 128])
                    sgn = hqb.tile([128, DT, 128], F16, tag="sgn")
                    nc.vector.tensor_scalar(sgn[:], wt[:], hh[:], None, OP.is_gt)
                    qng = hqb.tile([128, DT, 128], F16, tag="qng")
                    nc.vector.scalar_tensor_tensor(qng[:], wt[:], nhh[:], sgn[:],
                                                   OP.is_lt, OP.subtract)
                    o = osb.tile([128, NCORES, T], F16, tag="osb")
                    for r in range(NCORES):
                        pool = ps_h if r < 5 else ps_mm
                        pt = pool.tile([128, T], F32, tag="ps_mm")
                        for dt in range(DT):
                            nc.tensor.matmul(pt[:], qng[:, dt, :], acts[:, r, dt, :],
                                             start=(dt == 0), stop=(dt == DT - 1))
                        nc.scalar.activation(o[:, r, :], pt[:], AF.Identity,
                                             bias=hbs[:, g:g + 1], scale=snh[:])
                    nc.scalar.dma_start(
                        out_d.ap()[g * 128:(g + 1) * 128, :],
                        o[:].rearrange("p r t -> p (r t)"))

    nc.compile()
    return nc


_BUILD_CACHE = {}


def _get_nc(cfg: Cfg):
    key = (cfg.G,)
    if key not in _BUILD_CACHE:
        _BUILD_CACHE[key] = build(cfg)
    return _BUILD_CACHE[key]


def make_in_maps(cfg: Cfg, x, emb, w0, b0, w1, b1, ln_gamma, ln_beta, head_w, head_b):
    """Host-side sharding/layout prep. Returns list of per-core input dicts."""
    embh = np.asarray(emb, np.float32).astype(np.float16)
    w0t = np.ascontiguousarray(np.asarray(w0, np.float32).T)
    w1t = np.ascontiguousarray(np.asarray(w1, np.float32).T)
    hwt = np.ascontiguousarray(np.asarray(head_w, np.float32).T)  # [D, V]

    def rearr(v, n):
        return np.ascontiguousarray(np.asarray(v, np.float32).reshape(n, 128).T)

    b0r = rearr(b0, DT)
    b1r = rearr(b1, DT)
    gamr = rearr(ln_gamma, DT)
    betr = rearr(ln_beta, DT)
    hb = np.asarray(head_b, np.float32)

    ids = np.asarray(x).reshape(-1).astype(np.int16)
    assert ids.size == NCORES * T
    in_maps = []
    for c in range(NCORES):
        # indices wrapped into 16 partitions, replicated across the 8 Q7 stripes
        idx_arr = np.tile(ids[c * T:(c + 1) * T].reshape(T // 16, 16).T, (8, 1))
        lo, cnt = _START[c] * 128, _CNT[c] * 128
        wslh = np.zeros((DIM, HT * 128), np.float32)
        wslh[:, :cnt] = hwt[:, lo:lo + cnt]
        hbr = np.zeros((HT * 128,), np.float32)
        hbr[:cnt] = hb[lo:lo + cnt]
        in_maps.append(dict(
            idx=idx_arr, embh=embh, w0t=w0t, w1t=w1t,
            wsl0=np.ascontiguousarray(w0t[:, c * SL:(c + 1) * SL]),
            wsl1=np.ascontiguousarray(w1t[:, c * SL:(c + 1) * SL]),
            wslh=wslh, b0r=b0r, b1r=b1r, gamr=gamr, betr=betr,
            hbr=rearr(hbr, HT)))
    return in_maps


def _run(cfg: Cfg, inputs, trace=False):
    nc = _get_nc(cfg)
    in_maps = make_in_maps(cfg, **inputs)
    res = run_bass_kernel_spmd(nc, in_maps, core_ids=list(range(NCORES)),
                               trace=trace)
    full = np.empty((TF, VOCAB), np.float32)
    for c in range(NCORES):
        o = res.results[c]["out"].reshape(HT * 128, TF)
        lo, cnt = _START[c] * 128, _CNT[c] * 128
        full[:, lo:lo + cnt] = o[:cnt].T
    return full, res


def kernel(**inputs) -> np.ndarray:
    cfg = Cfg()
    full, _ = _run(cfg, inputs)
    return full.reshape(BATCH, SEQ, VOCAB)
